# revision 4
# baseline (speedup 1.0000x reference)
"""CapsuleLayer (dynamic routing) Trainium2 kernel, v2.

Problem: B=128, I=1152 input capsules (A=8), O=10 output capsules (OA=16),
3 routing iterations.  Data-parallel over batch: 8 cores x 16 examples.

Per-core layout: SBUF partition p = is*16 + b  (is = i mod 8, b = local
batch), chunk c = i // 8 in the free dim, vote coordinate n = oa*10 + o
(o innermost so softmax/squash reductions are innermost-axis reductions).

v2 changes vs v1:
  - phase-1 PSUM->SBUF vote copies split DVE/Scalar so the PE matmul
    stream is not copy-starved; finer input DMA pieces.
  - single activation table set (natural_log_exp_and_others): sqrt is
    computed as exp(0.5*ln(x)); no ACT_TABLE_LOAD swaps.
  - logits kept in bf16 (2x DVE mode for the adds / exp input).
  - routing transitions (delta mult -> oa-pair-tree -> logits -> softmax
    -> wv -> next s-matmuls) issued per chunk-group so DVE / Scalar /
    GPSIMD / PE pipeline; GPSIMD takes a slice of the big elementwise.
  - squash chain trimmed: adds read s straight from PSUM, v cast fused.
"""

import numpy as np
import ml_dtypes

B, I, A, O, OA = 128, 1152, 8, 10, 16
NCORES = 8
BL = B // NCORES        # 16 examples per core
IS8 = 8                 # i-positions per half-chunk
C = I // IS8            # 144 half-chunks
CP = C // 2             # 72 paired chunks
N = O * OA              # 160, n = oa*O + o
N2 = 2 * N              # 320 per paired chunk
P = 128                 # p = is*BL + b
NUM_ROUTING = 3

GRP = 3                 # paired chunks per psum tile in phase 1
SLOT = 512              # psum bank-aligned slot (f32)
NG1 = CP // GRP         # 24 phase-1 groups
NDMA = 12               # input DMA pieces
SW = 3                  # half-chunks per s-matmul
Q = 3                   # chunk-groups per routing transition
CG = C // Q             # 48 half-chunks per transition group
GQ = 7                  # gpsimd-owned chunks at the tail of each group

_NC_CACHE = {}


def _build_nc():
    from contextlib import ExitStack

    import concourse.tile as tile
    import concourse.mybir as mybir
    from concourse import bacc

    F32 = mybir.dt.float32
    BF16 = mybir.dt.bfloat16
    AF = mybir.ActivationFunctionType
    ALU = mybir.AluOpType
    AX = mybir.AxisListType

    nc = bacc.Bacc()
    xbd_d = nc.dram_tensor("xbd", [P, CP, P], BF16, kind="ExternalInput")
    w2c_d = nc.dram_tensor("w2c", [P, CP, N2], BF16, kind="ExternalInput")
    bsel_d = nc.dram_tensor("bsel", [P, BL], BF16, kind="ExternalInput")
    brep_d = nc.dram_tensor("brep", [BL, P], BF16, kind="ExternalInput")
    bias_d = nc.dram_tensor("biasr", [BL, N], F32, kind="ExternalInput")
    vout_d = nc.dram_tensor("vout", [BL, N], F32, kind="ExternalOutput")

    with ExitStack() as ctx:
        tc = ctx.enter_context(tile.TileContext(nc))
        st = ctx.enter_context(tc.tile_pool(name="static", bufs=1))
        itp = ctx.enter_context(tc.tile_pool(name="itp", bufs=1))

        votes = st.tile([P, C, N], BF16)
        logits = st.tile([P, C, O], BF16)
        big = st.tile([P, C, N], BF16)      # shared: delta tmp / wv
        bsel = st.tile([P, BL], BF16)
        brep = st.tile([BL, P], BF16)
        biasr = st.tile([BL, N], F32)

        nc.sync.dma_start(out=bsel[:], in_=bsel_d[:])
        nc.sync.dma_start(out=brep[:], in_=brep_d[:])
        nc.sync.dma_start(out=biasr[:], in_=bias_d[:])

        # ---- phase 1: votes ----
        with tc.tile_pool(name="ph1", bufs=1) as ph1, tc.tile_pool(
            name="psv", bufs=2, space="PSUM"
        ) as psv:
            xbd = ph1.tile([P, CP, P], BF16)
            w2c = ph1.tile([P, CP, N2], BF16)
            dstep = CP // NDMA
            for q in range(NDMA):
                sl = slice(q * dstep, (q + 1) * dstep)
                nc.sync.dma_start(out=xbd[:, sl, :], in_=xbd_d[:, sl, :])
                nc.sync.dma_start(out=w2c[:, sl, :], in_=w2c_d[:, sl, :])
            for g in range(NG1):
                ps = psv.tile([P, GRP * SLOT], F32, tag="pv")
                for j in range(GRP):
                    cp = g * GRP + j
                    nc.tensor.matmul(
                        ps[:, j * SLOT : j * SLOT + N2],
                        lhsT=xbd[:, cp, :],
                        rhs=w2c[:, cp, :],
                        start=True,
                        stop=True,
                    )
                src = ps[:].rearrange("p (j s) -> p j s", j=GRP)[:, :, 0:N2]
                dst = votes[:, g * 2 * GRP : (g + 1) * 2 * GRP, :].rearrange(
                    "p (j c2) n -> p j (c2 n)", j=GRP
                )
                if g % 3 == 2:
                    nc.scalar.copy(dst, src)
                else:
                    nc.vector.tensor_copy(dst, src)

        # ---- routing ----
        pss = ctx.enter_context(tc.tile_pool(name="pss", bufs=1, space="PSUM"))
        NS = C // SW

        expb = itp.tile([P, C, O], BF16, tag="expb")
        zf = itp.tile([P, C], F32, tag="z")
        rz = itp.tile([P, C], F32, tag="rz")
        route = itp.tile([P, C, O], BF16, tag="route")
        dd = itp.tile([P, C, O], BF16, tag="dd")

        s_ps = {}
        for t in range(1, NUM_ROUTING + 1):
            sps_t = pss.tile([BL, SW * N], F32, tag=f"sps{t}")
            s_ps[t] = sps_t

        # s-matmuls for t=1 straight from votes
        for j in range(NS):
            rhs = votes[:, j * SW : (j + 1) * SW, :].rearrange("p c n -> p (c n)")
            nc.tensor.matmul(
                s_ps[1][:], lhsT=bsel[:], rhs=rhs, start=(j == 0), stop=(j == NS - 1)
            )

        def squash(t):
            """s_ps[t] -> v (vbf bf16 for t<3, vt f32 for t=3), then vrep."""
            s3 = itp.tile([BL, SW, N], F32, tag="s3")
            nc.scalar.copy(s3[:], s_ps[t][:].rearrange("b (c n) -> b c n", c=SW))
            sa = itp.tile([BL, N], F32, tag="sa")
            nc.vector.tensor_add(sa[:], s3[:, 0, :], s3[:, 1, :])
            s_t = itp.tile([BL, N], F32, tag="stile")
            if t == 1:
                # s = (sa + s3[2]) * (1/O) + bias
                nc.vector.tensor_add(sa[:], sa[:], s3[:, 2, :])
                nc.vector.scalar_tensor_tensor(
                    s_t[:], sa[:], 1.0 / O, biasr[:], op0=ALU.mult, op1=ALU.add
                )
            else:
                nc.vector.tensor_add(sa[:], sa[:], s3[:, 2, :])
                nc.vector.tensor_add(s_t[:], sa[:], biasr[:])

            sq = itp.tile([BL, N], F32, tag="sq")
            nc.vector.tensor_mul(sq[:], s_t[:], s_t[:])
            nsq = itp.tile([BL, OA], F32, tag="nsq")
            nc.vector.reduce_sum(
                nsq[:], sq[:].rearrange("b (oa o) -> b oa o", o=O), axis=AX.X
            )
            nsq1 = itp.tile([BL, OA], F32, tag="nsq1")
            nc.vector.tensor_scalar_add(nsq1[:], nsq[:], 1.0)
            rn1 = itp.tile([BL, OA], F32, tag="rn1")
            nc.vector.reciprocal_approx_fast(rn1[:], nsq1[:])
            # sqrt(nsq) = exp(0.5*ln(nsq)) -- stays in one act table set
            lnn = itp.tile([BL, OA], F32, tag="lnn")
            nc.scalar.activation(lnn[:], nsq[:], AF.Ln)
            sr = itp.tile([BL, OA], F32, tag="sr")
            nc.scalar.activation(sr[:], lnn[:], AF.Exp, scale=0.5)
            f = itp.tile([BL, OA], F32, tag="f")
            nc.vector.tensor_mul(f[:], sr[:], rn1[:])
            f_b = f[:].unsqueeze(2).broadcast_to([BL, OA, O])
            s3d = s_t[:].rearrange("b (oa o) -> b oa o", o=O)
            if t == NUM_ROUTING:
                vt = itp.tile([BL, N], F32, tag="vt")
                nc.vector.tensor_mul(vt[:].rearrange("b (oa o) -> b oa o", o=O), s3d, f_b)
                nc.sync.dma_start(out=vout_d[:], in_=vt[:])
                return None
            vbf = itp.tile([BL, N], BF16, tag="vbf")
            nc.vector.tensor_mul(vbf[:].rearrange("b (oa o) -> b oa o", o=O), s3d, f_b)
            vr_ps = pss.tile([P, N], F32, tag=f"vrps{t}")
            nc.tensor.matmul(vr_ps[:], lhsT=brep[:], rhs=vbf[:], start=True, stop=True)
            vrep = itp.tile([P, N], BF16, tag=f"vrep{t}")
            nc.scalar.copy(vrep[:], vr_ps[:])
            return vrep

        big4 = big[:].rearrange("p c (oa o) -> p c oa o", o=O)
        v4 = votes[:].rearrange("p c (oa o) -> p c oa o", o=O)

        for t in range(1, NUM_ROUTING + 1):
            vrep = squash(t)
            if t == NUM_ROUTING:
                break
            vr_b = vrep[:].unsqueeze(1).broadcast_to([P, C, N])
            # transition t -> t+1, per chunk-group
            for q in range(Q):
                c0, c1 = q * CG, (q + 1) * CG
                cd = c1 - GQ  # DVE / gpsimd split point
                # tmp = votes * vrep  (delta elementwise)
                nc.vector.tensor_mul(big[:, c0:cd], votes[:, c0:cd], vr_b[:, c0:cd])
                if GQ:
                    nc.gpsimd.tensor_mul(big[:, cd:c1], votes[:, cd:c1], vr_b[:, cd:c1])
                # pair-tree over oa: 16 -> 2
                for h in (8, 4, 2):
                    nc.vector.tensor_add(
                        big4[:, c0:cd, 0:h, :],
                        big4[:, c0:cd, 0:h, :],
                        big4[:, c0:cd, h : 2 * h, :],
                    )
                    if GQ:
                        nc.gpsimd.tensor_add(
                            big4[:, cd:c1, 0:h, :],
                            big4[:, cd:c1, 0:h, :],
                            big4[:, cd:c1, h : 2 * h, :],
                        )
                # logits update (bf16)
                if t == 1:
                    nc.vector.tensor_add(
                        logits[:, c0:c1], big4[:, c0:c1, 0, :], big4[:, c0:c1, 1, :]
                    )
                else:
                    nc.vector.tensor_add(
                        dd[:, c0:c1], big4[:, c0:c1, 0, :], big4[:, c0:c1, 1, :]
                    )
                    nc.vector.tensor_add(logits[:, c0:c1], logits[:, c0:c1], dd[:, c0:c1])
                # softmax over o
                nc.scalar.activation(expb[:, c0:c1], logits[:, c0:c1], AF.Exp)
                nc.vector.reduce_sum(zf[:, c0:c1], expb[:, c0:c1], axis=AX.X)
                nc.vector.reciprocal_approx_fast(rz[:, c0:c1], zf[:, c0:c1])
                nc.vector.tensor_mul(
                    route[:, c0:c1],
                    expb[:, c0:c1],
                    rz[:, c0:c1].unsqueeze(2).broadcast_to([P, c1 - c0, O]),
                )
                # wv = route * votes (overwrites tmp region)
                r4 = route[:].unsqueeze(2).broadcast_to([P, C, OA, O])
                nc.vector.tensor_mul(big4[:, c0:cd], v4[:, c0:cd], r4[:, c0:cd])
                if GQ:
                    nc.gpsimd.tensor_mul(big4[:, cd:c1], v4[:, cd:c1], r4[:, cd:c1])
                # s-matmuls for t+1 over this group's chunks
                for j in range(c0 // SW, c1 // SW):
                    rhs = big[:, j * SW : (j + 1) * SW, :].rearrange("p c n -> p (c n)")
                    nc.tensor.matmul(
                        s_ps[t + 1][:],
                        lhsT=bsel[:],
                        rhs=rhs,
                        start=(j == 0),
                        stop=(j == NS - 1),
                    )

    nc.compile()
    return nc


def get_nc():
    if "nc" not in _NC_CACHE:
        _NC_CACHE["nc"] = _build_nc()
    return _NC_CACHE["nc"]


def make_in_maps(x, weights, biases):
    bf = ml_dtypes.bfloat16
    x = np.asarray(x, np.float32)
    weights = np.asarray(weights, np.float32)
    biases = np.asarray(biases, np.float32)

    # w2c[(h, is, a), cp, h2*N + (oa, o)] = w[(2cp+h)*8+is, a, o*16+oa] * (h==h2)
    w5 = (
        weights.reshape(CP, 2, IS8, A, O, OA)
        .transpose(0, 1, 2, 3, 5, 4)
        .reshape(CP, 2, IS8, A, N)
    )
    w2c = np.zeros((CP, 2, IS8, A, 2, N), np.float32)
    for h in range(2):
        w2c[:, h, :, :, h, :] = w5[:, h]
    w2c = w2c.reshape(CP, P, N2).transpose(1, 0, 2).astype(bf)

    eye = np.eye(BL, dtype=np.float32)
    bsel = np.tile(eye, (IS8, 1)).astype(bf)  # bsel[p, b'] = delta(p % BL == b')
    brep = np.tile(eye, (1, IS8)).astype(bf)  # brep[b, p] = delta(b == p % BL)
    biasr = np.broadcast_to(biases.T.reshape(1, N), (BL, N)).astype(np.float32).copy()

    in_maps = []
    idx = np.arange(IS8)
    for k in range(NCORES):
        xc = x[k * BL : (k + 1) * BL]  # [BL, I, A]
        xt = xc.reshape(BL, C, IS8, A).transpose(2, 1, 3, 0)  # [IS8, C, A, BL]
        xbd = np.zeros((C, IS8, A, IS8, BL), np.float32)
        # LHS advanced-index result shape: [IS8, C, A, BL]; RHS xt matches.
        xbd[:, idx, :, idx, :] = xt
        # [C=2*CP, (is,a)=64, (is',b)=128] -> pair chunks into k=128
        xbd = xbd.reshape(CP, 2 * IS8 * A, IS8 * BL).transpose(1, 0, 2).astype(bf)
        in_maps.append(
            {
                "xbd": np.ascontiguousarray(xbd),
                "w2c": w2c,
                "bsel": bsel,
                "brep": brep,
                "biasr": biasr,
            }
        )
    return in_maps


def assemble_out(results):
    out = np.zeros((B, 1, O, OA), np.float32)
    for k in range(NCORES):
        v = np.asarray(results[k]["vout"], np.float32)  # [BL, N], n = oa*O + o
        out[k * BL : (k + 1) * BL, 0] = v.reshape(BL, OA, O).transpose(0, 2, 1)
    return out


def kernel(x, weights, biases):
    from concourse.bass_utils import run_bass_kernel_spmd

    nc = get_nc()
    in_maps = make_in_maps(x, weights, biases)
    res = run_bass_kernel_spmd(nc, in_maps, list(range(NCORES)))
    return assemble_out(res.results)


# revision 6
# speedup vs baseline: 1.2103x; 1.2103x over previous
"""CapsuleLayer (dynamic routing) Trainium2 kernel, v2.

Problem: B=128, I=1152 input capsules (A=8), O=10 output capsules (OA=16),
3 routing iterations.  Data-parallel over batch: 8 cores x 16 examples.

Per-core layout: SBUF partition p = is*16 + b  (is = i mod 8, b = local
batch), chunk c = i // 8 in the free dim, vote coordinate n = oa*10 + o
(o innermost so softmax/squash reductions are innermost-axis reductions).

v2 changes vs v1:
  - phase-1 PSUM->SBUF vote copies split DVE/Scalar so the PE matmul
    stream is not copy-starved; finer input DMA pieces.
  - single activation table set (natural_log_exp_and_others): sqrt is
    computed as exp(0.5*ln(x)); no ACT_TABLE_LOAD swaps.
  - logits kept in bf16 (2x DVE mode for the adds / exp input).
  - routing transitions (delta mult -> oa-pair-tree -> logits -> softmax
    -> wv -> next s-matmuls) issued per chunk-group so DVE / Scalar /
    GPSIMD / PE pipeline; GPSIMD takes a slice of the big elementwise.
  - squash chain trimmed: adds read s straight from PSUM, v cast fused.
"""

import numpy as np
import ml_dtypes

B, I, A, O, OA = 128, 1152, 8, 10, 16
NCORES = 8
BL = B // NCORES        # 16 examples per core
IS8 = 8                 # i-positions per half-chunk
C = I // IS8            # 144 half-chunks
CP = C // 2             # 72 paired chunks
N = O * OA              # 160, n = oa*O + o
N2 = 2 * N              # 320 per paired chunk
P = 128                 # p = is*BL + b
NUM_ROUTING = 3

GRP = 3                 # paired chunks per psum tile in phase 1
SLOT = 512              # psum bank-aligned slot (f32)
NG1 = CP // GRP         # 24 phase-1 groups
NDMA = 12               # input DMA pieces
SW = 3                  # half-chunks per s-matmul
Q = 3                   # chunk-groups per routing transition
CG = C // Q             # 48 half-chunks per transition group
GQ = 0                  # gpsimd-owned chunks at the tail of each group (0: off)

_NC_CACHE = {}


def _patch_act_tables():
    """Force all Exp/Ln/Copy activations into natural_log_exp_and_others.

    The table-load pass binds each activation to the first set containing
    its function, which thrashes between exp_and_others and natural_log.
    Strip Exp/Ln from every other set (index-preserving) so one table-set
    serves the whole kernel.
    """
    import concourse.bacc as bacc_mod
    import concourse.mybir as mybir

    if getattr(bacc_mod, "_capsule_act_patch", False):
        return
    orig = bacc_mod.get_activation_tables
    pref = "natural_log_exp_and_others"
    strip = {
        mybir.ActivationFunctionType.Exp,
        mybir.ActivationFunctionType.Ln,
    }

    def patched(arch):
        t = orig(arch)
        if pref not in t:
            return t
        return {k: (v if k == pref else (v - strip)) for k, v in t.items()}

    patched.__wrapped__ = orig
    bacc_mod.get_activation_tables = patched
    bacc_mod._capsule_act_patch = True


def _build_nc():
    from contextlib import ExitStack

    import concourse.tile as tile
    import concourse.mybir as mybir
    from concourse import bacc

    _patch_act_tables()

    F32 = mybir.dt.float32
    BF16 = mybir.dt.bfloat16
    AF = mybir.ActivationFunctionType
    ALU = mybir.AluOpType
    AX = mybir.AxisListType

    nc = bacc.Bacc()
    xbd_d = nc.dram_tensor("xbd", [P, CP, P], BF16, kind="ExternalInput")
    w2c_d = nc.dram_tensor("w2c", [P, CP, N2], BF16, kind="ExternalInput")
    bsel_d = nc.dram_tensor("bsel", [P, BL], BF16, kind="ExternalInput")
    brep_d = nc.dram_tensor("brep", [BL, P], BF16, kind="ExternalInput")
    bias_d = nc.dram_tensor("biasr", [BL, N], F32, kind="ExternalInput")
    vout_d = nc.dram_tensor("vout", [BL, N], F32, kind="ExternalOutput")

    with ExitStack() as ctx:
        tc = ctx.enter_context(tile.TileContext(nc))
        st = ctx.enter_context(tc.tile_pool(name="static", bufs=1))
        itp = ctx.enter_context(tc.tile_pool(name="itp", bufs=1))

        votes = st.tile([P, C, N], BF16)
        logits = st.tile([P, C, O], BF16)
        big = st.tile([P, C, N], BF16)      # shared: delta tmp / wv
        bsel = st.tile([P, BL], BF16)
        brep = st.tile([BL, P], BF16)
        biasr = st.tile([BL, N], F32)

        nc.sync.dma_start(out=bsel[:], in_=bsel_d[:])
        nc.sync.dma_start(out=brep[:], in_=brep_d[:])
        nc.sync.dma_start(out=biasr[:], in_=bias_d[:])

        # ---- phase 1: votes ----
        with tc.tile_pool(name="ph1", bufs=1) as ph1, tc.tile_pool(
            name="psv", bufs=2, space="PSUM"
        ) as psv:
            xbd = ph1.tile([P, CP, P], BF16)
            w2c = ph1.tile([P, CP, N2], BF16)
            dstep = CP // NDMA
            for q in range(NDMA):
                sl = slice(q * dstep, (q + 1) * dstep)
                nc.sync.dma_start(out=xbd[:, sl, :], in_=xbd_d[:, sl, :])
                nc.sync.dma_start(out=w2c[:, sl, :], in_=w2c_d[:, sl, :])
            for g in range(NG1):
                ps = psv.tile([P, GRP * SLOT], F32, tag="pv")
                for j in range(GRP):
                    cp = g * GRP + j
                    nc.tensor.matmul(
                        ps[:, j * SLOT : j * SLOT + N2],
                        lhsT=xbd[:, cp, :],
                        rhs=w2c[:, cp, :],
                        start=True,
                        stop=True,
                    )
                src = ps[:].rearrange("p (j s) -> p j s", j=GRP)[:, :, 0:N2]
                dst = votes[:, g * 2 * GRP : (g + 1) * 2 * GRP, :].rearrange(
                    "p (j c2) n -> p j (c2 n)", j=GRP
                )
                if g % 2 == 1:
                    nc.scalar.copy(dst, src)
                else:
                    nc.vector.tensor_copy(dst, src)

        # ---- routing ----
        pss = ctx.enter_context(tc.tile_pool(name="pss", bufs=1, space="PSUM"))
        NS = C // SW

        expb = itp.tile([P, C, O], BF16, tag="expb")
        zf = itp.tile([P, C], F32, tag="z")
        rz = itp.tile([P, C], F32, tag="rz")
        route = itp.tile([P, C, O], BF16, tag="route")
        dd = itp.tile([P, C, O], BF16, tag="dd")

        s_ps = {}
        for t in range(1, NUM_ROUTING + 1):
            sps_t = pss.tile([BL, SW * N], F32, tag=f"sps{t}")
            s_ps[t] = sps_t

        # s-matmuls for t=1 straight from votes
        for j in range(NS):
            rhs = votes[:, j * SW : (j + 1) * SW, :].rearrange("p c n -> p (c n)")
            nc.tensor.matmul(
                s_ps[1][:], lhsT=bsel[:], rhs=rhs, start=(j == 0), stop=(j == NS - 1)
            )

        def squash(t):
            """s_ps[t] -> v (vbf bf16 for t<3, vt f32 for t=3), then vrep."""
            s3 = itp.tile([BL, SW, N], F32, tag="s3")
            nc.scalar.copy(s3[:], s_ps[t][:].rearrange("b (c n) -> b c n", c=SW))
            sa = itp.tile([BL, N], F32, tag="sa")
            nc.vector.tensor_add(sa[:], s3[:, 0, :], s3[:, 1, :])
            s_t = itp.tile([BL, N], F32, tag="stile")
            if t == 1:
                # s = (sa + s3[2]) * (1/O) + bias
                nc.vector.tensor_add(sa[:], sa[:], s3[:, 2, :])
                nc.vector.scalar_tensor_tensor(
                    s_t[:], sa[:], 1.0 / O, biasr[:], op0=ALU.mult, op1=ALU.add
                )
            else:
                nc.vector.tensor_add(sa[:], sa[:], s3[:, 2, :])
                nc.vector.tensor_add(s_t[:], sa[:], biasr[:])

            sq = itp.tile([BL, N], F32, tag="sq")
            nc.vector.tensor_mul(sq[:], s_t[:], s_t[:])
            nsq = itp.tile([BL, OA], F32, tag="nsq")
            nc.vector.reduce_sum(
                nsq[:], sq[:].rearrange("b (oa o) -> b oa o", o=O), axis=AX.X
            )
            nsq1 = itp.tile([BL, OA], F32, tag="nsq1")
            nc.vector.tensor_scalar_add(nsq1[:], nsq[:], 1.0)
            rn1 = itp.tile([BL, OA], F32, tag="rn1")
            nc.vector.reciprocal_approx_fast(rn1[:], nsq1[:])
            # sqrt(nsq) = exp(0.5*ln(nsq)) -- stays in one act table set
            lnn = itp.tile([BL, OA], F32, tag="lnn")
            nc.scalar.activation(lnn[:], nsq[:], AF.Ln)
            sr = itp.tile([BL, OA], F32, tag="sr")
            nc.scalar.activation(sr[:], lnn[:], AF.Exp, scale=0.5)
            f = itp.tile([BL, OA], F32, tag="f")
            nc.vector.tensor_mul(f[:], sr[:], rn1[:])
            f_b = f[:].unsqueeze(2).broadcast_to([BL, OA, O])
            s3d = s_t[:].rearrange("b (oa o) -> b oa o", o=O)
            if t == NUM_ROUTING:
                vt = itp.tile([BL, N], F32, tag="vt")
                nc.vector.tensor_mul(vt[:].rearrange("b (oa o) -> b oa o", o=O), s3d, f_b)
                nc.sync.dma_start(out=vout_d[:], in_=vt[:])
                return None
            vbf = itp.tile([BL, N], BF16, tag="vbf")
            nc.vector.tensor_mul(vbf[:].rearrange("b (oa o) -> b oa o", o=O), s3d, f_b)
            vr_ps = pss.tile([P, N], F32, tag=f"vrps{t}")
            nc.tensor.matmul(vr_ps[:], lhsT=brep[:], rhs=vbf[:], start=True, stop=True)
            vrep = itp.tile([P, N], BF16, tag=f"vrep{t}")
            nc.scalar.copy(vrep[:], vr_ps[:])
            return vrep

        big4 = big[:].rearrange("p c (oa o) -> p c oa o", o=O)
        v4 = votes[:].rearrange("p c (oa o) -> p c oa o", o=O)

        for t in range(1, NUM_ROUTING + 1):
            vrep = squash(t)
            if t == NUM_ROUTING:
                break
            vr_b = vrep[:].unsqueeze(1).broadcast_to([P, C, N])
            # transition t -> t+1, per chunk-group
            for q in range(Q):
                c0, c1 = q * CG, (q + 1) * CG
                cd = c1 - GQ  # DVE / gpsimd split point
                # tmp = votes * vrep  (delta elementwise)
                nc.vector.tensor_mul(big[:, c0:cd], votes[:, c0:cd], vr_b[:, c0:cd])
                if GQ:
                    nc.gpsimd.tensor_mul(big[:, cd:c1], votes[:, cd:c1], vr_b[:, cd:c1])
                # pair-tree over oa: 16 -> 2
                for h in (8, 4, 2):
                    nc.vector.tensor_add(
                        big4[:, c0:cd, 0:h, :],
                        big4[:, c0:cd, 0:h, :],
                        big4[:, c0:cd, h : 2 * h, :],
                    )
                    if GQ:
                        nc.gpsimd.tensor_add(
                            big4[:, cd:c1, 0:h, :],
                            big4[:, cd:c1, 0:h, :],
                            big4[:, cd:c1, h : 2 * h, :],
                        )
                # logits update (bf16)
                if t == 1:
                    nc.vector.tensor_add(
                        logits[:, c0:c1], big4[:, c0:c1, 0, :], big4[:, c0:c1, 1, :]
                    )
                else:
                    nc.vector.tensor_add(
                        dd[:, c0:c1], big4[:, c0:c1, 0, :], big4[:, c0:c1, 1, :]
                    )
                    nc.vector.tensor_add(logits[:, c0:c1], logits[:, c0:c1], dd[:, c0:c1])
                # softmax over o
                nc.scalar.activation(expb[:, c0:c1], logits[:, c0:c1], AF.Exp)
                nc.vector.reduce_sum(zf[:, c0:c1], expb[:, c0:c1], axis=AX.X)
                nc.vector.reciprocal_approx_fast(rz[:, c0:c1], zf[:, c0:c1])
                nc.vector.tensor_mul(
                    route[:, c0:c1],
                    expb[:, c0:c1],
                    rz[:, c0:c1].unsqueeze(2).broadcast_to([P, c1 - c0, O]),
                )
                # wv = route * votes (overwrites tmp region)
                r4 = route[:].unsqueeze(2).broadcast_to([P, C, OA, O])
                nc.vector.tensor_mul(big4[:, c0:cd], v4[:, c0:cd], r4[:, c0:cd])
                if GQ:
                    nc.gpsimd.tensor_mul(big4[:, cd:c1], v4[:, cd:c1], r4[:, cd:c1])
                # s-matmuls for t+1 over this group's chunks
                for j in range(c0 // SW, c1 // SW):
                    rhs = big[:, j * SW : (j + 1) * SW, :].rearrange("p c n -> p (c n)")
                    nc.tensor.matmul(
                        s_ps[t + 1][:],
                        lhsT=bsel[:],
                        rhs=rhs,
                        start=(j == 0),
                        stop=(j == NS - 1),
                    )

    nc.compile()
    return nc


def get_nc():
    if "nc" not in _NC_CACHE:
        _NC_CACHE["nc"] = _build_nc()
    return _NC_CACHE["nc"]


def make_in_maps(x, weights, biases):
    bf = ml_dtypes.bfloat16
    x = np.asarray(x, np.float32)
    weights = np.asarray(weights, np.float32)
    biases = np.asarray(biases, np.float32)

    # w2c[(h, is, a), cp, h2*N + (oa, o)] = w[(2cp+h)*8+is, a, o*16+oa] * (h==h2)
    w5 = (
        weights.reshape(CP, 2, IS8, A, O, OA)
        .transpose(0, 1, 2, 3, 5, 4)
        .reshape(CP, 2, IS8, A, N)
    )
    w2c = np.zeros((CP, 2, IS8, A, 2, N), np.float32)
    for h in range(2):
        w2c[:, h, :, :, h, :] = w5[:, h]
    w2c = w2c.reshape(CP, P, N2).transpose(1, 0, 2).astype(bf)

    eye = np.eye(BL, dtype=np.float32)
    bsel = np.tile(eye, (IS8, 1)).astype(bf)  # bsel[p, b'] = delta(p % BL == b')
    brep = np.tile(eye, (1, IS8)).astype(bf)  # brep[b, p] = delta(b == p % BL)
    biasr = np.broadcast_to(biases.T.reshape(1, N), (BL, N)).astype(np.float32).copy()

    in_maps = []
    idx = np.arange(IS8)
    for k in range(NCORES):
        xc = x[k * BL : (k + 1) * BL]  # [BL, I, A]
        xt = xc.reshape(BL, C, IS8, A).transpose(2, 1, 3, 0)  # [IS8, C, A, BL]
        xbd = np.zeros((C, IS8, A, IS8, BL), np.float32)
        # LHS advanced-index result shape: [IS8, C, A, BL]; RHS xt matches.
        xbd[:, idx, :, idx, :] = xt
        # [C=2*CP, (is,a)=64, (is',b)=128] -> pair chunks into k=128
        xbd = xbd.reshape(CP, 2 * IS8 * A, IS8 * BL).transpose(1, 0, 2).astype(bf)
        in_maps.append(
            {
                "xbd": np.ascontiguousarray(xbd),
                "w2c": w2c,
                "bsel": bsel,
                "brep": brep,
                "biasr": biasr,
            }
        )
    return in_maps


def assemble_out(results):
    out = np.zeros((B, 1, O, OA), np.float32)
    for k in range(NCORES):
        v = np.asarray(results[k]["vout"], np.float32)  # [BL, N], n = oa*O + o
        out[k * BL : (k + 1) * BL, 0] = v.reshape(BL, OA, O).transpose(0, 2, 1)
    return out


def kernel(x, weights, biases):
    from concourse.bass_utils import run_bass_kernel_spmd

    nc = get_nc()
    in_maps = make_in_maps(x, weights, biases)
    res = run_bass_kernel_spmd(nc, in_maps, list(range(NCORES)))
    return assemble_out(res.results)


# revision 7
# speedup vs baseline: 1.2371x; 1.0221x over previous
"""CapsuleLayer (dynamic routing) Trainium2 kernel, v4.

Problem: B=128, I=1152 input capsules (A=8), O=10 output capsules (OA=16),
3 routing iterations.  Data-parallel over batch: 8 cores x 16 examples.

Per-core layout: SBUF partition p = is*16 + b  (is = i mod 8, b = local
batch), chunk c = i // 8 in the free dim, vote coordinate n = oa*10 + o
(o innermost so softmax/squash reductions are innermost-axis reductions).

Key scheduling decisions (see git history for the measured evolution):
  - phase-1 PSUM->SBUF vote copies split DVE/Scalar (they are the phase-1
    pacer); input DMA issued from both Sync and Scalar queues with small
    priming pieces so the PE starts early.
  - single activation table set (natural_log_exp_and_others): sqrt is
    exp(0.5*ln(x)); Copy/Exp/Ln are stripped from competing sets so no
    ACT_TABLE_LOAD swaps occur mid-kernel.
  - logits kept in bf16 (2x DVE mode); bias folded into the s-matmul
    accumulation (extra n=160 matmul); t=1's 1/O route folded into a
    scaled bsel.
  - delta (mult + oa-pair-tree) in few big DVE ops, chunked ~3.5us with
    PE heartbeat matmuls after each piece so the PE's HAM governor never
    sees a >3.4us idle window (cold PE doubles s-matmul time).
  - softmax/wv split per chunk-group so Scalar exp / DVE / PE s-matmuls
    pipeline; final transition uses finer tail groups.
"""

import numpy as np
import ml_dtypes

B, I, A, O, OA = 128, 1152, 8, 10, 16
NCORES = 8
BL = B // NCORES        # 16 examples per core
IS8 = 8                 # i-positions per half-chunk
C = I // IS8            # 144 half-chunks
CP = C // 2             # 72 paired chunks
N = O * OA              # 160, n = oa*O + o
N2 = 2 * N              # 320 per paired chunk
P = 128                 # p = is*BL + b
NUM_ROUTING = 3

GRP = 3                 # paired chunks per psum tile in phase 1
SLOT = 512              # psum bank-aligned slot (f32)
NG1 = CP // GRP         # 24 phase-1 groups
SW = 3                  # half-chunks per s-matmul
NS = C // SW            # 48 s-matmuls per iteration
DMA_PIECES = [2, 2, 3, 3, 5, 5, 7, 7, 9, 9, 10, 10]  # cp per input piece

_NC_CACHE = {}


def _patch_act_tables():
    """Bind all Copy/Exp/Ln activations to natural_log_exp_and_others.

    The table-load pass binds each activation to the first set containing
    its function, which thrashes between sets.  Strip Copy/Exp/Ln from
    every other set (index-preserving) so one table-set serves the whole
    kernel and only one ACT_TABLE_LOAD is emitted.
    """
    import concourse.bacc as bacc_mod
    import concourse.mybir as mybir

    if getattr(bacc_mod, "_capsule_act_patch", False):
        return
    orig = bacc_mod.get_activation_tables
    pref = "natural_log_exp_and_others"
    strip = {
        mybir.ActivationFunctionType.Exp,
        mybir.ActivationFunctionType.Ln,
        mybir.ActivationFunctionType.Copy,
    }

    def patched(arch):
        t = orig(arch)
        if pref not in t:
            return t
        return {k: (v if k == pref else (v - strip)) for k, v in t.items()}

    patched.__wrapped__ = orig
    bacc_mod.get_activation_tables = patched
    bacc_mod._capsule_act_patch = True


def _build_nc():
    from contextlib import ExitStack

    import concourse.tile as tile
    import concourse.mybir as mybir
    from concourse import bacc

    _patch_act_tables()

    F32 = mybir.dt.float32
    BF16 = mybir.dt.bfloat16
    AF = mybir.ActivationFunctionType
    ALU = mybir.AluOpType
    AX = mybir.AxisListType

    nc = bacc.Bacc()
    xbd_d = nc.dram_tensor("xbd", [P, CP, P], BF16, kind="ExternalInput")
    w2c_d = nc.dram_tensor("w2c", [P, CP, N2], BF16, kind="ExternalInput")
    bsel_d = nc.dram_tensor("bsel", [P, BL], BF16, kind="ExternalInput")
    bsel1_d = nc.dram_tensor("bsel1", [P, BL], BF16, kind="ExternalInput")
    brep_d = nc.dram_tensor("brep", [BL, P], BF16, kind="ExternalInput")
    brow_d = nc.dram_tensor("brow", [P, N], BF16, kind="ExternalInput")
    vout_d = nc.dram_tensor("vout", [BL, N], F32, kind="ExternalOutput")

    with ExitStack() as ctx:
        tc = ctx.enter_context(tile.TileContext(nc))
        st = ctx.enter_context(tc.tile_pool(name="static", bufs=1))
        itp = ctx.enter_context(tc.tile_pool(name="itp", bufs=1))

        votes = st.tile([P, C, N], BF16)
        logits = st.tile([P, C, O], BF16)
        big = st.tile([P, C, N], BF16)      # shared: delta tmp / wv
        bsel = st.tile([P, BL], BF16)
        bsel1 = st.tile([P, BL], BF16)
        brep = st.tile([BL, P], BF16)
        brow = st.tile([P, N], BF16)

        nc.sync.dma_start(out=bsel[:], in_=bsel_d[:])
        nc.sync.dma_start(out=bsel1[:], in_=bsel1_d[:])
        nc.sync.dma_start(out=brep[:], in_=brep_d[:])
        nc.sync.dma_start(out=brow[:], in_=brow_d[:])

        # ---- phase 1: votes ----
        with tc.tile_pool(name="ph1", bufs=1) as ph1, tc.tile_pool(
            name="psv", bufs=2, space="PSUM"
        ) as psv:
            xbd = ph1.tile([P, CP, P], BF16)
            w2c = ph1.tile([P, CP, N2], BF16)
            off = 0
            for sz in DMA_PIECES:
                sl = slice(off, off + sz)
                nc.sync.dma_start(out=xbd[:, sl, :], in_=xbd_d[:, sl, :])
                nc.scalar.dma_start(out=w2c[:, sl, :], in_=w2c_d[:, sl, :])
                off += sz
            for g in range(NG1):
                ps = psv.tile([P, GRP * SLOT], F32, tag="pv")
                for j in range(GRP):
                    cp = g * GRP + j
                    nc.tensor.matmul(
                        ps[:, j * SLOT : j * SLOT + N2],
                        lhsT=xbd[:, cp, :],
                        rhs=w2c[:, cp, :],
                        start=True,
                        stop=True,
                    )
                src = ps[:].rearrange("p (j s) -> p j s", j=GRP)[:, :, 0:N2]
                dst = votes[:, g * 2 * GRP : (g + 1) * 2 * GRP, :].rearrange(
                    "p (j c2) n -> p j (c2 n)", j=GRP
                )
                if g % 2 == 1 and g >= 4:
                    nc.scalar.copy(dst, src)
                else:
                    nc.vector.tensor_copy(dst, src)

        # ---- routing ----
        pss = ctx.enter_context(tc.tile_pool(name="pss", bufs=1, space="PSUM"))

        expb = itp.tile([P, C, O], BF16, tag="expb")
        zf = itp.tile([P, C], F32, tag="z")
        rz = itp.tile([P, C], F32, tag="rz")
        route = itp.tile([P, C, O], BF16, tag="route")
        dd = itp.tile([P, C, O], BF16, tag="dd")
        hb_ps = pss.tile([BL, 32], F32, tag="hb")

        s_ps = {}
        for t in range(1, NUM_ROUTING + 1):
            sps_t = pss.tile([BL, SW * N], F32, tag=f"sps{t}")
            s_ps[t] = sps_t

        def s_matmuls(t, dst_ps, src, j0, j1):
            """Accumulating s-matmuls for iteration t over chunk range."""
            lhs = bsel1 if t == 1 else bsel
            for j in range(j0, j1):
                rhs = src[:, j * SW : (j + 1) * SW, :].rearrange("p c n -> p (c n)")
                nc.tensor.matmul(
                    dst_ps[:], lhsT=lhs, rhs=rhs, start=(j == 0), stop=False
                )
            if j1 == NS:
                # bias fold: bsel.T @ brow adds biasr into the first piece
                nc.tensor.matmul(
                    dst_ps[:, 0:N], lhsT=bsel[:], rhs=brow[:], start=False, stop=True
                )

        def heartbeat(region):
            # tiny matmul keeping the PE's HAM activity window warm; depends
            # on a freshly-written SBUF region so it fires mid-DVE-stream
            nc.tensor.matmul(
                hb_ps[:], lhsT=bsel[:], rhs=region, start=True, stop=True
            )

        s_matmuls(1, s_ps[1], votes, 0, NS)

        def squash(t):
            """s_ps[t] -> v (vbf bf16 for t<3, vt f32 for t=3), then vrep."""
            s3 = itp.tile([BL, SW, N], F32, tag="s3")
            nc.scalar.copy(s3[:], s_ps[t][:].rearrange("b (c n) -> b c n", c=SW))
            sa = itp.tile([BL, N], F32, tag="sa")
            nc.vector.tensor_add(sa[:], s3[:, 0, :], s3[:, 1, :])
            s_t = itp.tile([BL, N], F32, tag="stile")
            nc.vector.tensor_add(s_t[:], sa[:], s3[:, 2, :])

            sq = itp.tile([BL, N], F32, tag="sq")
            nc.vector.tensor_mul(sq[:], s_t[:], s_t[:])
            nsq = itp.tile([BL, OA], F32, tag="nsq")
            nc.vector.reduce_sum(
                nsq[:], sq[:].rearrange("b (oa o) -> b oa o", o=O), axis=AX.X
            )
            nsq1 = itp.tile([BL, OA], F32, tag="nsq1")
            nc.vector.tensor_scalar_add(nsq1[:], nsq[:], 1.0)
            rn1 = itp.tile([BL, OA], F32, tag="rn1")
            nc.vector.reciprocal_approx_fast(rn1[:], nsq1[:])
            # sqrt(nsq) = exp(0.5*ln(nsq)) -- stays in one act table set
            lnn = itp.tile([BL, OA], F32, tag="lnn")
            nc.scalar.activation(lnn[:], nsq[:], AF.Ln)
            sr = itp.tile([BL, OA], F32, tag="sr")
            nc.scalar.activation(sr[:], lnn[:], AF.Exp, scale=0.5)
            f = itp.tile([BL, OA], F32, tag="f")
            nc.vector.tensor_mul(f[:], sr[:], rn1[:])
            f_b = f[:].unsqueeze(2).broadcast_to([BL, OA, O])
            s3d = s_t[:].rearrange("b (oa o) -> b oa o", o=O)
            if t == NUM_ROUTING:
                vt = itp.tile([BL, N], F32, tag="vt")
                nc.vector.tensor_mul(vt[:].rearrange("b (oa o) -> b oa o", o=O), s3d, f_b)
                nc.sync.dma_start(out=vout_d[:], in_=vt[:])
                return None
            vbf = itp.tile([BL, N], BF16, tag="vbf")
            nc.vector.tensor_mul(vbf[:].rearrange("b (oa o) -> b oa o", o=O), s3d, f_b)
            vr_ps = pss.tile([P, N], F32, tag=f"vrps{t}")
            nc.tensor.matmul(vr_ps[:], lhsT=brep[:], rhs=vbf[:], start=True, stop=True)
            vrep = itp.tile([P, N], BF16, tag=f"vrep{t}")
            nc.scalar.copy(vrep[:], vr_ps[:])
            return vrep

        big4 = big[:].rearrange("p c (oa o) -> p c oa o", o=O)
        v4 = votes[:].rearrange("p c (oa o) -> p c oa o", o=O)

        for t in range(1, NUM_ROUTING + 1):
            vrep = squash(t)
            if t == NUM_ROUTING:
                break
            vr_b = vrep[:].unsqueeze(1).broadcast_to([P, C, N])

            # delta: tmp = votes*vrep, in ~3.5us pieces with PE heartbeats
            for k in range(4):
                c0, c1 = k * (C // 4), (k + 1) * (C // 4)
                nc.vector.tensor_mul(big[:, c0:c1], votes[:, c0:c1], vr_b[:, c0:c1])
                heartbeat(big[:, c1 - 1, 0:32])
            # pair-tree over oa: 16 -> 2
            for h in (8, 4, 2):
                for k in range(2):
                    c0, c1 = k * (C // 2), (k + 1) * (C // 2)
                    nc.vector.tensor_add(
                        big4[:, c0:c1, 0:h, :],
                        big4[:, c0:c1, 0:h, :],
                        big4[:, c0:c1, h : 2 * h, :],
                    )
                    heartbeat(big[:, c1 - 1, 0:32])
            # logits update (bf16)
            if t == 1:
                nc.vector.tensor_add(logits[:], big4[:, :, 0, :], big4[:, :, 1, :])
            else:
                nc.vector.tensor_add(dd[:], big4[:, :, 0, :], big4[:, :, 1, :])
                nc.vector.tensor_add(logits[:], logits[:], dd[:])

            # softmax + wv per chunk-group (finer tail on last transition)
            groups = [48, 48, 48] if t == 1 else [48, 48, 24, 24]
            c0 = 0
            for gsz in groups:
                c1 = c0 + gsz
                nc.scalar.activation(expb[:, c0:c1], logits[:, c0:c1], AF.Exp)
                nc.vector.reduce_sum(zf[:, c0:c1], expb[:, c0:c1], axis=AX.X)
                nc.vector.reciprocal_approx_fast(rz[:, c0:c1], zf[:, c0:c1])
                nc.vector.tensor_mul(
                    route[:, c0:c1],
                    expb[:, c0:c1],
                    rz[:, c0:c1].unsqueeze(2).broadcast_to([P, gsz, O]),
                )
                r4 = route[:].unsqueeze(2).broadcast_to([P, C, OA, O])
                nc.vector.tensor_mul(big4[:, c0:c1], v4[:, c0:c1], r4[:, c0:c1])
                s_matmuls(t + 1, s_ps[t + 1], big, c0 // SW, c1 // SW)
                c0 = c1

    nc.compile()
    return nc


def get_nc():
    if "nc" not in _NC_CACHE:
        _NC_CACHE["nc"] = _build_nc()
    return _NC_CACHE["nc"]


def make_in_maps(x, weights, biases):
    bf = ml_dtypes.bfloat16
    x = np.asarray(x, np.float32)
    weights = np.asarray(weights, np.float32)
    biases = np.asarray(biases, np.float32)

    # w2c[(h, is, a), cp, h2*N + (oa, o)] = w[(2cp+h)*8+is, a, o*16+oa] * (h==h2)
    w5 = (
        weights.reshape(CP, 2, IS8, A, O, OA)
        .transpose(0, 1, 2, 3, 5, 4)
        .reshape(CP, 2, IS8, A, N)
    )
    w2c = np.zeros((CP, 2, IS8, A, 2, N), np.float32)
    for h in range(2):
        w2c[:, h, :, :, h, :] = w5[:, h]
    w2c = w2c.reshape(CP, P, N2).transpose(1, 0, 2).astype(bf)

    eye = np.eye(BL, dtype=np.float32)
    bsel = np.tile(eye, (IS8, 1))            # bsel[p, b'] = delta(p % BL == b')
    brep = np.tile(eye, (1, IS8)).astype(bf)  # brep[b, p] = delta(b == p % BL)
    # bias as a matmul operand: rows 0..BL-1 hold biasr, rest zero
    brow = np.zeros((P, N), np.float32)
    brow[:BL] = biases.T.reshape(1, N)
    brow = brow.astype(bf)

    in_maps = []
    idx = np.arange(IS8)
    for k in range(NCORES):
        xc = x[k * BL : (k + 1) * BL]  # [BL, I, A]
        xt = xc.reshape(BL, C, IS8, A).transpose(2, 1, 3, 0)  # [IS8, C, A, BL]
        xbd = np.zeros((C, IS8, A, IS8, BL), np.float32)
        # LHS advanced-index result shape: [IS8, C, A, BL]; RHS xt matches.
        xbd[:, idx, :, idx, :] = xt
        # [C=2*CP, (is,a)=64, (is',b)=128] -> pair chunks into k=128
        xbd = xbd.reshape(CP, 2 * IS8 * A, IS8 * BL).transpose(1, 0, 2).astype(bf)
        in_maps.append(
            {
                "xbd": np.ascontiguousarray(xbd),
                "w2c": w2c,
                "bsel": bsel.astype(bf),
                "bsel1": (bsel / O).astype(bf),
                "brep": brep,
                "brow": brow,
            }
        )
    return in_maps


def assemble_out(results):
    out = np.zeros((B, 1, O, OA), np.float32)
    for k in range(NCORES):
        v = np.asarray(results[k]["vout"], np.float32)  # [BL, N], n = oa*O + o
        out[k * BL : (k + 1) * BL, 0] = v.reshape(BL, OA, O).transpose(0, 2, 1)
    return out


def kernel(x, weights, biases):
    from concourse.bass_utils import run_bass_kernel_spmd

    nc = get_nc()
    in_maps = make_in_maps(x, weights, biases)
    res = run_bass_kernel_spmd(nc, in_maps, list(range(NCORES)))
    return assemble_out(res.results)
